# revision 1
# baseline (speedup 1.0000x reference)
"""Trainium2 Bass kernel for an 8-expert top-2 MoE layer (B=4, T=2048, C=1024,
F=4096), expert-parallel across 8 NeuronCores.

Strategy
--------
The reference module is a *dense* MoE: it runs every expert's FFN on every
token, then combines with top-2 gate weights — so 6 of 8 expert outputs per
token are multiplied by zero.  The output only depends on each token's top-2
experts, so we route: the host computes the (tiny) gate in fp32, assigns each
token to its two experts, and the device computes each expert's FFN over just
the tokens routed to it.  The host then scatter-adds the gate-weighted
per-expert outputs.

The gate MUST be computed in fp32: the smallest 2nd-vs-3rd expert logit margin
over the 8192 tokens is ~3.6e-5, and a bf16 gate flips the selected expert set
for ~17 tokens, each flip producing an O(1) relative error at that token.  The
fp32 host gate matches the reference selection with a ~20x margin.

Load balancing: expert token counts vary (~1930..2180), and an SPMD program
pads every core to the busiest expert.  We instead pair a big expert with a
small one (sorted largest<->smallest) and split each pair's FFN across two
cores along the F axis: core 2p+h runs BOTH experts of pair p over F-half h.
Per-core work becomes (n_big + n_small)/2 full-F-equivalents, i.e. the pair
average instead of the global max.  The two cores' partial outputs (each a
full [n, C] sum over its F-half; b2 is pre-halved on the host so the halves
sum to one b2) are added on the host during the scatter.

On-device math per core (pair p, F-half h), for each expert e in the pair:
    hT[f, t]   = sum_c W1[c, f] * xT[c, t]        (PE, bf16 inputs, fp32 acc)
    hT         = gelu_erf(hT + b1[f])             (ScalarE, fused bias)
    out[t, cc] = sum_{f in half} h[t, f] * W2[f, cc]   (PE, bf16 h, fp32 acc)
    out        = out + b2/2                       (VectorE, fp32)
Computing h in transposed form (tokens in the free dim) is what lets the
second matmul contract over F without any on-device transpose.
"""

import os

import numpy as np
import ml_dtypes

import concourse.bass as bass
import concourse.mybir as mybir
import concourse.tile as tile
from concourse import bacc
from concourse.bass_utils import run_bass_kernel_spmd

C = 1024
F = 4096
FH = F // 2  # per-core F half
E = 8
K = 2
N_CORES = 8
CHUNK = 512

BF16 = mybir.dt.bfloat16
F32 = mybir.dt.float32


def build_nc(chunks_a: list[int], chunks_b: list[int]) -> bass.Bass:
    """Bass program: two experts' FFNs (F-half depth) over their token chunks.

    chunks_a/chunks_b: per-chunk token counts for expert slot A / B,
    each 0 < ch <= 512.
    """
    nta, ntb = sum(chunks_a), sum(chunks_b)
    assert all(0 < ch <= 512 for ch in chunks_a + chunks_b)
    nc = bacc.Bacc(None)

    # inputs: token stream and weights for expert slots A and B
    xta = nc.dram_tensor("xta", [C, nta], BF16, kind="ExternalInput")
    xtb = nc.dram_tensor("xtb", [C, ntb], BF16, kind="ExternalInput")
    w1a = nc.dram_tensor("w1a", [C, FH], BF16, kind="ExternalInput")
    w1b = nc.dram_tensor("w1b", [C, FH], BF16, kind="ExternalInput")
    w2a = nc.dram_tensor("w2a", [FH, C], BF16, kind="ExternalInput")
    w2b = nc.dram_tensor("w2b", [FH, C], BF16, kind="ExternalInput")
    # b1t[s][p, j] = b1[slot s][(j*128)+p] for this core's F-half (j: f-tile)
    b1t = nc.dram_tensor("b1t", [2, 128, FH // 128], F32, kind="ExternalInput")
    # b2h[s] = b2[slot s] / 2, broadcast over partitions on device
    b2h = nc.dram_tensor("b2h", [2, C], F32, kind="ExternalInput")
    outa = nc.dram_tensor("outa", [nta, C], F32, kind="ExternalOutput")
    outb = nc.dram_tensor("outb", [ntb, C], F32, kind="ExternalOutput")

    n_ct = C // 128  # 8 contraction tiles for x @ W1
    n_ft = FH // 128  # 16 F tiles per half
    n_cc = C // 512  # 2 output column chunks

    with tile.TileContext(nc) as tc:
        with (
            tc.tile_pool(name="wpool", bufs=1) as wpool,
            tc.tile_pool(name="xpool", bufs=3) as xpool,
            tc.tile_pool(name="hpool", bufs=n_ft + 2) as hpool,
            tc.tile_pool(name="opool", bufs=4) as opool,
            tc.tile_pool(name="phpool", bufs=4, space="PSUM") as phpool,
            tc.tile_pool(name="popool", bufs=4, space="PSUM") as popool,
        ):
            # DMA issue order: biases first (tiny; the first gelu blocks PSUM
            # recycling on b1), chunk-0 activations, W1a in quarters (f-tile
            # order), then W2a / slot-B tensors which are needed later.
            b1_sb = wpool.tile([128, 2, n_ft], F32, name="b1sb", tag="b1sb")
            nc.sync.dma_start(out=b1_sb[:, 0, :], in_=b1t[0])
            nc.sync.dma_start(out=b1_sb[:, 1, :], in_=b1t[1])
            b2_sb = wpool.tile([128, 2, C], F32, name="b2sb", tag="b2sb")
            for s in range(2):
                nc.sync.dma_start(
                    out=b2_sb[:, s, :], in_=b2h[s : s + 1, :].to_broadcast([128, C])
                )

            xts0 = []
            for c in range(n_ct):
                t = xpool.tile([128, chunks_a[0]], BF16, name=f"xta_0_{c}", tag=f"xt{c}")
                nc.sync.dma_start(out=t, in_=xta[c * 128 : (c + 1) * 128, : chunks_a[0]])
                xts0.append(t)

            w1_sb = {
                s: [
                    wpool.tile([128, FH], BF16, name=f"w1sb{s}_{c}", tag=f"w1sb{s}_{c}")
                    for c in range(n_ct)
                ]
                for s in range(2)
            }
            w2_sb = {
                s: [
                    wpool.tile([128, C], BF16, name=f"w2sb{s}_{f}", tag=f"w2sb{s}_{f}")
                    for f in range(n_ft)
                ]
                for s in range(2)
            }
            # slot-A weights up front: W1a in quarters (f-tile order), then W2a
            for quarter in range(4):
                fs = slice(quarter * (FH // 4), (quarter + 1) * (FH // 4))
                for c in range(n_ct):
                    nc.sync.dma_start(
                        out=w1_sb[0][c][:, fs], in_=w1a[c * 128 : (c + 1) * 128, fs]
                    )
            for f in range(n_ft):
                nc.sync.dma_start(out=w2_sb[0][f], in_=w2a[f * 128 : (f + 1) * 128, :])

            # slot-B weight loads, spread between slot-A chunks so they don't
            # starve the slot-A activation streams in the DMA queues.
            deferred_loads = [
                [
                    lambda c=c: nc.sync.dma_start(
                        out=w1_sb[1][c], in_=w1b[c * 128 : (c + 1) * 128, :]
                    )
                    for c in range(n_ct)
                ],
                [
                    lambda f=f: nc.sync.dma_start(
                        out=w2_sb[1][f], in_=w2b[f * 128 : (f + 1) * 128, :]
                    )
                    for f in range(n_ft // 2)
                ],
                [
                    lambda f=f: nc.sync.dma_start(
                        out=w2_sb[1][f], in_=w2b[f * 128 : (f + 1) * 128, :]
                    )
                    for f in range(n_ft // 2, n_ft)
                ],
            ]

            def run_slot(s, xtd, outd, chunks, first_xts):
                tok0 = 0
                for tk, ch in enumerate(chunks):
                    if first_xts is not None and tk == 0:
                        xts = first_xts
                    else:
                        xts = []
                        for c in range(n_ct):
                            t = xpool.tile(
                                [128, ch], BF16, name=f"xt{s}_{tk}_{c}", tag=f"xt{c}"
                            )
                            nc.sync.dma_start(
                                out=t,
                                in_=xtd[c * 128 : (c + 1) * 128, tok0 : tok0 + ch],
                            )
                            xts.append(t)
                    if s == 0 and tk >= 1 and deferred_loads:
                        for emit in deferred_loads.pop(0):
                            emit()

                    hts = []
                    for f in range(n_ft):
                        ph = phpool.tile([128, ch], F32, name=f"ph{s}_{tk}_{f}", tag="ph")
                        for c in range(n_ct):
                            nc.tensor.matmul(
                                ph,
                                lhsT=w1_sb[s][c][:, f * 128 : (f + 1) * 128],
                                rhs=xts[c],
                                start=(c == 0),
                                stop=(c == n_ct - 1),
                            )
                        ht = hpool.tile([128, ch], BF16, name=f"ht{s}_{tk}_{f}", tag="ht")
                        nc.scalar.activation(
                            out=ht,
                            in_=ph,
                            func=mybir.ActivationFunctionType.Gelu,
                            bias=b1_sb[:, s, f : f + 1],
                            scale=1.0,
                        )
                        hts.append(ht)

                    for tt in range((ch + 127) // 128):
                        tw = min(128, ch - tt * 128)
                        for cc in range(n_cc):
                            po = popool.tile(
                                [128, 512], F32, name=f"po{s}_{tk}_{tt}_{cc}", tag="po"
                            )
                            for f in range(n_ft):
                                nc.tensor.matmul(
                                    po[:tw, :],
                                    lhsT=hts[f][:, tt * 128 : tt * 128 + tw],
                                    rhs=w2_sb[s][f][:, cc * 512 : (cc + 1) * 512],
                                    start=(f == 0),
                                    stop=(f == n_ft - 1),
                                )
                            ot = opool.tile(
                                [128, 512], F32, name=f"ot{s}_{tk}_{tt}_{cc}", tag="ot"
                            )
                            nc.vector.tensor_add(
                                ot[:tw, :],
                                po[:tw, :],
                                b2_sb[:tw, s, cc * 512 : (cc + 1) * 512],
                            )
                            r0 = tok0 + tt * 128
                            nc.sync.dma_start(
                                out=outd[r0 : r0 + tw, cc * 512 : (cc + 1) * 512],
                                in_=ot[:tw, :],
                            )
                    tok0 += ch

            run_slot(0, xta, outa, chunks_a, xts0)
            while deferred_loads:  # in case slot A had very few chunks
                for emit in deferred_loads.pop(0):
                    emit()
            run_slot(1, xtb, outb, chunks_b, None)
    nc.finalize()
    return nc


def pick_chunks(n: int) -> list[int]:
    """[512]*a + [exact tail] — matmul N needs no alignment."""
    n512 = n // 512
    rem = n - n512 * 512
    chunks = [512] * n512
    if rem > 0:
        chunks.append(rem)
    if not chunks:
        chunks = [1]
    return chunks


def _route(x2d: np.ndarray, Wg: np.ndarray):
    """fp32 gate identical in selection to the reference; returns per-expert
    token indices and renormalized top-2 weights."""
    logits = x2d @ Wg  # fp32 BLAS
    order = np.argsort(-logits, axis=1, kind="stable")
    top2 = order[:, :K]  # [N, 2]
    m = logits.max(axis=1, keepdims=True)
    p = np.exp(logits - m, dtype=np.float32)
    p /= p.sum(axis=1, keepdims=True)
    tw = np.take_along_axis(p, top2, axis=1)
    tw /= tw.sum(axis=1, keepdims=True)  # [N, 2] renormalized
    idxs, ws = [], []
    for e in range(E):
        sel = top2 == e  # [N, 2] bool, at most one True per row
        rows = np.where(sel.any(axis=1))[0]
        idxs.append(rows)
        ws.append(tw[rows][sel[rows]])
    return idxs, ws


_LAST_RESULTS = {}  # stash for test harness introspection (exec time etc.)


def kernel(**inputs: np.ndarray) -> np.ndarray:
    x = np.asarray(inputs["x"], dtype=np.float32)
    Wg = np.asarray(inputs["Wg"], dtype=np.float32)
    W1 = np.asarray(inputs["W1"], dtype=np.float32)
    b1 = np.asarray(inputs["b1"], dtype=np.float32)
    W2 = np.asarray(inputs["W2"], dtype=np.float32)
    b2 = np.asarray(inputs["b2"], dtype=np.float32)

    B, T, Cx = x.shape
    assert Cx == C
    x2d = np.ascontiguousarray(x.reshape(-1, C))
    n_tok_total = x2d.shape[0]

    idxs, ws = _route(x2d, Wg)
    counts = np.array([len(i) for i in idxs])

    # Pair the largest expert with the smallest, 2nd largest with 2nd
    # smallest, etc.  Pair p runs on cores 2p (F-half 0) and 2p+1 (F-half 1).
    order = np.argsort(-counts, kind="stable")
    pairs = [(int(order[p]), int(order[E - 1 - p])) for p in range(E // 2)]
    nta = max(counts[a] for a, _ in pairs)
    ntb = max(counts[b] for _, b in pairs)
    chunks_a = pick_chunks(int(nta))
    chunks_b = pick_chunks(int(ntb))
    nta, ntb = sum(chunks_a), sum(chunks_b)

    w1h = W1.astype(ml_dtypes.bfloat16)  # [E, C, F]
    w2h = W2.astype(ml_dtypes.bfloat16)  # [E, F, C]

    def xt_for(e, ntok):
        xe = np.zeros((ntok, C), dtype=np.float32)
        xe[: counts[e]] = x2d[idxs[e]]
        return np.ascontiguousarray(xe.T).astype(ml_dtypes.bfloat16)

    xt_cache = {}
    for a, b_ in pairs:
        xt_cache[a] = xt_for(a, nta)
        xt_cache[b_] = xt_for(b_, ntb)

    in_maps = []
    for core in range(N_CORES):
        p, h = divmod(core, 2)
        ea, eb = pairs[p]
        fsl = slice(h * FH, (h + 1) * FH)
        b1t = np.stack(
            [
                np.ascontiguousarray(b1[ea][fsl].reshape(FH // 128, 128).T),
                np.ascontiguousarray(b1[eb][fsl].reshape(FH // 128, 128).T),
            ]
        ).astype(np.float32)
        in_maps.append(
            {
                "xta": xt_cache[ea],
                "xtb": xt_cache[eb],
                "w1a": np.ascontiguousarray(w1h[ea][:, fsl]),
                "w1b": np.ascontiguousarray(w1h[eb][:, fsl]),
                "w2a": np.ascontiguousarray(w2h[ea][fsl, :]),
                "w2b": np.ascontiguousarray(w2h[eb][fsl, :]),
                "b1t": b1t,
                "b2h": np.stack([b2[ea], b2[eb]]).astype(np.float32) * 0.5,
            }
        )

    nc = build_nc(chunks_a, chunks_b)
    trace = os.environ.get("KERNEL_TRACE", "") == "1"
    res = run_bass_kernel_spmd(
        nc, in_maps, core_ids=list(range(N_CORES)), trace=trace
    )
    _LAST_RESULTS["bass_results"] = res
    if trace and res.exec_time_ns is not None:
        print(f"[kernel] HW exec time: {res.exec_time_ns} ns")

    out = np.zeros((n_tok_total, C), dtype=np.float32)
    for p, (ea, eb) in enumerate(pairs):
        for e, key in ((ea, "outa"), (eb, "outb")):
            n_e = counts[e]
            oe = (
                np.asarray(res.results[2 * p][key])[:n_e]
                + np.asarray(res.results[2 * p + 1][key])[:n_e]
            )
            out[idxs[e]] += ws[e][:, None] * oe
    return out.reshape(B, T, C)



# revision 8
# speedup vs baseline: 1.0172x; 1.0172x over previous
"""Trainium2 Bass kernel for an 8-expert top-2 MoE layer (B=4, T=2048, C=1024,
F=4096), expert-parallel across 8 NeuronCores.

Strategy
--------
The reference module is a *dense* MoE: it runs every expert's FFN on every
token, then combines with top-2 gate weights — so 6 of 8 expert outputs per
token are multiplied by zero.  The output only depends on each token's top-2
experts, so we route: the host computes the (tiny) gate in fp32, assigns each
token to its two experts, and the device computes each expert's FFN over just
the tokens routed to it.  The host then scatter-adds the gate-weighted
per-expert outputs (plus the gate-weighted b2, which never goes to device).

The gate MUST be computed in fp32: the smallest 2nd-vs-3rd expert logit margin
over the 8192 tokens is ~3.6e-5, and a bf16 gate flips the selected expert set
for ~17 tokens, each flip producing an O(1) relative error at that token.

Load balancing: expert token counts vary (~1930..2180), and an SPMD program
pads every core to the busiest expert.  We pair a big expert with a small one
(sorted largest<->smallest) and split each pair's FFN across two cores along
the F axis: core 2p+h runs BOTH experts of pair p over F-half h.  The two
cores' partial outputs are summed on the host during the scatter.

On-device math per core (pair p, F-half h), for each expert e in the pair:
    hT[f, t]   = sum_c W1[c, f] * xT[c, t]        (PE, bf16 inputs, fp32 acc)
    hT         = gelu_erf(hT + b1[f])             (ScalarE, fused bias)
    out[t, cc] = sum_{f in half} h[t, f] * W2[f, cc]   (PE, bf16 h, fp32 acc)
out ships to HBM in bf16 (error contribution ~4e-4 rel, budget is 2e-2).

Schedule notes (from perfetto trace of the v1 kernel):
  * The NEFF preamble occupies ~6.4us; the PE steady state is ~99.6% busy at
    the warm N=512 issue rate (216 ns/MM), so the only recoverable time is
    startup (first MM was at 20.3us), the HAM cold-clock window, and the tail.
  * v2: ~80 tiny scratch matmuls issue at t~6.5us to warm the HAM clock gate
    while DMAs stream; the first token chunk is 256 wide and its weight/x
    DMAs are issued in exact consumption order as a few large transfers
    (x is shipped as [128, 8, nt] so a whole chunk is ONE strided DMA).
  * All DMA lands on one FIFO HW queue, so issue order == service order;
    big slot-B weight loads are spread between slot-A chunk loads.
"""

import os

import numpy as np
import ml_dtypes

import concourse.bass as bass
import concourse.mybir as mybir
import concourse.tile as tile
from concourse import bacc
from concourse.bass_utils import run_bass_kernel_spmd

C = 1024
F = 4096
FH = F // 2  # per-core F half
E = 8
K = 2
N_CORES = 8

BF16 = mybir.dt.bfloat16
F32 = mybir.dt.float32

N_CT = C // 128  # 8 contraction tiles for x @ W1
N_FT = FH // 128  # 16 F tiles per half
N_CC = C // 512  # 2 output column chunks


def build_nc(chunks_a: list[int], chunks_b: list[int]) -> bass.Bass:
    """Bass program: two experts' FFNs (F-half depth) over their token chunks."""
    nta, ntb = sum(chunks_a), sum(chunks_b)
    assert all(0 < ch <= 512 for ch in chunks_a + chunks_b)
    nc = bacc.Bacc(None)

    # x shipped pre-swizzled as [128, 8, nt]: (p, chi, t) = xT[chi*128 + p, t]
    # x0a duplicates chunk 0 of slot A contiguously: strided 512B-run HBM
    # reads measured ~60GB/s, contiguous ~350GB/s — and chunk 0 is the
    # startup critical path.
    x0a = nc.dram_tensor("x0a", [128, N_CT, chunks_a[0]], BF16, kind="ExternalInput")
    xta = nc.dram_tensor("xta", [128, N_CT, nta], BF16, kind="ExternalInput")
    xtb = nc.dram_tensor("xtb", [128, N_CT, ntb], BF16, kind="ExternalInput")
    # W1 quarter-major [4, 128, 8, FH/4] so each startup piece is contiguous:
    # (q, p, chi, f) = W1[chi*128 + p, q*512 + f]
    w1a = nc.dram_tensor("w1a", [4, 128, N_CT, FH // 4], BF16, kind="ExternalInput")
    w1b = nc.dram_tensor("w1b", [4, 128, N_CT, FH // 4], BF16, kind="ExternalInput")
    # W2 half-major [2, 128, 8, C]: (h, p, fhi, c) = W2[(8h + fhi)*128 + p, c]
    w2a = nc.dram_tensor("w2a", [2, 128, N_FT // 2, C], BF16, kind="ExternalInput")
    w2b = nc.dram_tensor("w2b", [2, 128, N_FT // 2, C], BF16, kind="ExternalInput")
    # b1t[s][p, j] = b1[slot s][(j*128)+p] for this core's F-half (j: f-tile)
    b1t = nc.dram_tensor("b1t", [2, 128, N_FT], F32, kind="ExternalInput")
    outa = nc.dram_tensor("outa", [nta, C], BF16, kind="ExternalOutput")
    outb = nc.dram_tensor("outb", [ntb, C], BF16, kind="ExternalOutput")

    with tile.TileContext(nc) as tc:
        with (
            tc.tile_pool(name="wpool", bufs=1) as wpool,
            tc.tile_pool(name="xpool", bufs=4) as xpool,
            tc.tile_pool(name="hpool", bufs=N_FT + 2) as hpool,
            tc.tile_pool(name="opool", bufs=4) as opool,
            tc.tile_pool(name="phpool", bufs=4, space="PSUM") as phpool,
            tc.tile_pool(name="popool", bufs=4, space="PSUM") as popool,
        ):
            # --- HAM warmup: ~100 tiny matmuls on a scratch tile keep the PE
            # busy from the end of the NEFF preamble so the clock gate opens
            # (K=8/8) before the first real matmul.  memset on GpSimd: its
            # preamble ends ~3us before DVE/PE's, so the PE never waits.
            warm_w = wpool.tile([128, 64], BF16, name="warm_w", tag="warmw")
            nc.gpsimd.memset(warm_w, 0)
            warm_ps = popool.tile([128, 64], F32, name="warm_ps", tag="po")
            for _ in range(100):
                nc.tensor.matmul(warm_ps[:64, :], lhsT=warm_w[:, :64], rhs=warm_w,
                                 start=True, stop=True)

            # --- SBUF weight/bias tiles
            w1_sb = {
                s: wpool.tile([128, N_CT, FH], BF16, name=f"w1sb{s}", tag=f"w1sb{s}")
                for s in range(2)
            }
            w2_sb = {
                s: wpool.tile([128, N_FT, C], BF16, name=f"w2sb{s}", tag=f"w2sb{s}")
                for s in range(2)
            }
            b1_sb = wpool.tile([128, 2, N_FT], F32, name="b1sb", tag="b1sb")

            # --- Startup DMAs in exact consumption order (single FIFO queue),
            # every piece a fully contiguous HBM read: chunk-0 (256 tokens)
            # runs DMA-paced right behind them.
            QF = FH // 4  # 512: f-columns per W1 quarter

            def load_w1(s, src, q):
                nc.sync.dma_start(
                    out=w1_sb[s][:, :, q * QF : (q + 1) * QF], in_=src[q]
                )

            def load_w2(s, src, h):
                nc.sync.dma_start(out=w2_sb[s][:, 8 * h : 8 * h + 8, :], in_=src[h])

            load_w1(0, w1a, 0)
            xts = {}
            xts[(0, 0)] = xpool.tile(
                [128, N_CT, chunks_a[0]], BF16, name="xta_0", tag="xt"
            )
            nc.sync.dma_start(out=xts[(0, 0)], in_=x0a[:, :, :])
            nc.sync.dma_start(out=b1_sb[:, 0, :], in_=b1t[0])
            nc.sync.dma_start(out=b1_sb[:, 1, :], in_=b1t[1])
            for q in (1, 2, 3):
                load_w1(0, w1a, q)
            load_w2(0, w2a, 0)
            load_w2(0, w2a, 1)

            # slot-B weight loads, spread between slot-A chunks so they don't
            # starve the slot-A activation stream in the FIFO DMA queue.
            deferred_loads = [
                [lambda: load_w1(1, w1b, 0), lambda: load_w1(1, w1b, 1)],
                [lambda: load_w1(1, w1b, 2), lambda: load_w1(1, w1b, 3)],
                [lambda: load_w2(1, w2b, 0)],
                [lambda: load_w2(1, w2b, 1)],
            ]

            def run_slot(s, xtd, outd, chunks):
                tok0 = 0
                for tk, ch in enumerate(chunks):
                    if (s, tk) in xts:
                        xt = xts[(s, tk)]
                    else:
                        xt = xpool.tile(
                            [128, N_CT, ch], BF16, name=f"xt{s}_{tk}", tag="xt"
                        )
                        nc.sync.dma_start(
                            out=xt, in_=xtd[:, :, tok0 : tok0 + ch]
                        )
                    if s == 0 and tk >= 1 and deferred_loads:
                        for emit in deferred_loads.pop(0):
                            emit()

                    hts = []
                    for f in range(N_FT):
                        ph = phpool.tile([128, ch], F32, name=f"ph{s}_{tk}_{f}", tag="ph")
                        for c in range(N_CT):
                            nc.tensor.matmul(
                                ph,
                                lhsT=w1_sb[s][:, c, f * 128 : (f + 1) * 128],
                                rhs=xt[:, c, :],
                                start=(c == 0),
                                stop=(c == N_CT - 1),
                            )
                        ht = hpool.tile([128, ch], BF16, name=f"ht{s}_{tk}_{f}", tag="ht")
                        nc.scalar.activation(
                            out=ht,
                            in_=ph,
                            func=mybir.ActivationFunctionType.Gelu,
                            bias=b1_sb[:, s, f : f + 1],
                            scale=1.0,
                        )
                        hts.append(ht)

                    for tt in range((ch + 127) // 128):
                        tw = min(128, ch - tt * 128)
                        for cc in range(N_CC):
                            po = popool.tile(
                                [128, 512], F32, name=f"po{s}_{tk}_{tt}_{cc}", tag="po"
                            )
                            for f in range(N_FT):
                                nc.tensor.matmul(
                                    po[:tw, :],
                                    lhsT=hts[f][:, tt * 128 : tt * 128 + tw],
                                    rhs=w2_sb[s][:, f, cc * 512 : (cc + 1) * 512],
                                    start=(f == 0),
                                    stop=(f == N_FT - 1),
                                )
                            ot = opool.tile(
                                [128, 512], BF16, name=f"ot{s}_{tk}_{tt}_{cc}", tag="ot"
                            )
                            nc.vector.tensor_copy(ot[:tw, :], po[:tw, :])
                            r0 = tok0 + tt * 128
                            nc.sync.dma_start(
                                out=outd[r0 : r0 + tw, cc * 512 : (cc + 1) * 512],
                                in_=ot[:tw, :],
                            )
                    tok0 += ch

            run_slot(0, xta, outa, chunks_a)
            while deferred_loads:  # in case slot A had very few chunks
                for emit in deferred_loads.pop(0):
                    emit()
            run_slot(1, xtb, outb, chunks_b)
    nc.finalize()
    return nc


def pick_chunks(n: int, small_first: bool) -> list[int]:
    """Chunks <=512 summing to n, minimizing sum(ceil(ch/128)) (MM2 tile
    count) while keeping every chunk >=~256 so MM1 is not LDWEIGHTS-bound.
    small_first: lead with a 256 chunk so the first chunk's DMAs are small."""
    if n <= 512:
        return [n]
    chunks = []
    rem = n
    if small_first and n > 768:
        chunks.append(256)
        rem -= 256
    while rem > 512 + 256:
        chunks.append(512)
        rem -= 512
    if rem > 512:
        chunks.append(rem // 2)
        rem -= rem // 2
    chunks.append(rem)
    return chunks


def _route(x2d: np.ndarray, Wg: np.ndarray):
    """fp32 gate identical in selection to the reference; returns per-expert
    token indices and renormalized top-2 weights."""
    logits = x2d @ Wg  # fp32 BLAS
    order = np.argsort(-logits, axis=1, kind="stable")
    top2 = order[:, :K]  # [N, 2]
    m = logits.max(axis=1, keepdims=True)
    p = np.exp(logits - m, dtype=np.float32)
    p /= p.sum(axis=1, keepdims=True)
    tw = np.take_along_axis(p, top2, axis=1)
    tw /= tw.sum(axis=1, keepdims=True)  # [N, 2] renormalized
    idxs, ws = [], []
    for e in range(E):
        sel = top2 == e  # [N, 2] bool, at most one True per row
        rows = np.where(sel.any(axis=1))[0]
        idxs.append(rows)
        ws.append(tw[rows][sel[rows]])
    return idxs, ws


_LAST_RESULTS = {}  # stash for test harness introspection (exec time etc.)


def kernel(**inputs: np.ndarray) -> np.ndarray:
    x = np.asarray(inputs["x"], dtype=np.float32)
    Wg = np.asarray(inputs["Wg"], dtype=np.float32)
    W1 = np.asarray(inputs["W1"], dtype=np.float32)
    b1 = np.asarray(inputs["b1"], dtype=np.float32)
    W2 = np.asarray(inputs["W2"], dtype=np.float32)
    b2 = np.asarray(inputs["b2"], dtype=np.float32)

    B, T, Cx = x.shape
    assert Cx == C
    x2d = np.ascontiguousarray(x.reshape(-1, C))
    n_tok_total = x2d.shape[0]

    idxs, ws = _route(x2d, Wg)
    counts = np.array([len(i) for i in idxs])

    # Pair the largest expert with the smallest, 2nd largest with 2nd
    # smallest, etc.  Pair p runs on cores 2p (F-half 0) and 2p+1 (F-half 1).
    order = np.argsort(-counts, kind="stable")
    pairs = [(int(order[p]), int(order[E - 1 - p])) for p in range(E // 2)]
    nta = max(counts[a] for a, _ in pairs)
    ntb = max(counts[b] for _, b in pairs)
    chunks_a = pick_chunks(int(nta), small_first=True)
    chunks_b = pick_chunks(int(ntb), small_first=False)
    nta, ntb = sum(chunks_a), sum(chunks_b)

    w1h = W1.astype(ml_dtypes.bfloat16)  # [E, C, F]
    w2h = W2.astype(ml_dtypes.bfloat16)  # [E, F, C]

    def xt_for(e, ntok):
        # [128, 8, ntok] with (p, chi, t) = x[t, chi*128 + p]
        xe = np.zeros((ntok, C), dtype=np.float32)
        xe[: counts[e]] = x2d[idxs[e]]
        xt = xe.T.reshape(N_CT, 128, ntok).transpose(1, 0, 2)
        return np.ascontiguousarray(xt).astype(ml_dtypes.bfloat16)

    xt_cache = {}
    for a, b_ in pairs:
        xt_cache[a] = xt_for(a, nta)
        xt_cache[b_] = xt_for(b_, ntb)

    in_maps = []
    for core in range(N_CORES):
        p, h = divmod(core, 2)
        ea, eb = pairs[p]
        fsl = slice(h * FH, (h + 1) * FH)
        b1t = np.stack(
            [
                np.ascontiguousarray(b1[ea][fsl].reshape(N_FT, 128).T),
                np.ascontiguousarray(b1[eb][fsl].reshape(N_FT, 128).T),
            ]
        ).astype(np.float32)

        def w1_lay(e):  # [4, 128, 8, FH/4]: (q, p, chi, f) = W1[e][chi*128+p, f0+q*512+f]
            w = w1h[e][:, fsl].reshape(N_CT, 128, 4, FH // 4).transpose(2, 1, 0, 3)
            return np.ascontiguousarray(w)

        def w2_lay(e):  # [2, 128, 8, C]: (h, p, fhi, c) = W2[e][f0+(8h+fhi)*128+p, c]
            w = w2h[e][fsl, :].reshape(2, N_FT // 2, 128, C).transpose(0, 2, 1, 3)
            return np.ascontiguousarray(w)

        in_maps.append(
            {
                "x0a": np.ascontiguousarray(xt_cache[ea][:, :, : chunks_a[0]]),
                "xta": xt_cache[ea],
                "xtb": xt_cache[eb],
                "w1a": w1_lay(ea),
                "w1b": w1_lay(eb),
                "w2a": w2_lay(ea),
                "w2b": w2_lay(eb),
                "b1t": b1t,
            }
        )

    nc = build_nc(chunks_a, chunks_b)
    trace = os.environ.get("KERNEL_TRACE", "") == "1"
    res = run_bass_kernel_spmd(
        nc, in_maps, core_ids=list(range(N_CORES)), trace=trace
    )
    _LAST_RESULTS["bass_results"] = res
    if trace and res.exec_time_ns is not None:
        print(f"[kernel] HW exec time: {res.exec_time_ns} ns")

    out = np.zeros((n_tok_total, C), dtype=np.float32)
    for p, (ea, eb) in enumerate(pairs):
        for e, key in ((ea, "outa"), (eb, "outb")):
            n_e = counts[e]
            oe = (
                np.asarray(res.results[2 * p][key])[:n_e].astype(np.float32)
                + np.asarray(res.results[2 * p + 1][key])[:n_e].astype(np.float32)
            )
            out[idxs[e]] += ws[e][:, None] * (oe + b2[e][None, :])
    return out.reshape(B, T, C)


# revision 11
# speedup vs baseline: 1.0369x; 1.0193x over previous
"""Trainium2 Bass kernel for an 8-expert top-2 MoE layer (B=4, T=2048, C=1024,
F=4096), expert-parallel across 8 NeuronCores.

Strategy
--------
The reference module is a *dense* MoE: it runs every expert's FFN on every
token, then combines with top-2 gate weights — so 6 of 8 expert outputs per
token are multiplied by zero.  The output only depends on each token's top-2
experts, so we route: the host computes the (tiny) gate in fp32, assigns each
token to its two experts, and the device computes each expert's FFN over just
the tokens routed to it.  The host then scatter-adds the gate-weighted
per-expert outputs (plus the gate-weighted b2, which never goes to device).

The gate MUST be computed in fp32: the smallest 2nd-vs-3rd expert logit margin
over the 8192 tokens is ~3.6e-5, and a bf16 gate flips the selected expert set
for ~17 tokens, each flip producing an O(1) relative error at that token.

Load balancing: expert token counts vary (~1930..2180), and an SPMD program
pads every core to the busiest expert.  We pair a big expert with a small one
(sorted largest<->smallest) and split each pair's FFN across two cores along
the F axis: core 2p+h runs BOTH experts of pair p over F-half h.  The two
cores' partial outputs are summed on the host during the scatter.

On-device math per core (pair p, F-half h), for each expert e in the pair:
    hT[f, t]   = sum_c W1[c, f] * xT[c, t]        (PE, bf16 inputs, fp32 acc)
    hT         = gelu_erf(hT + b1[f])             (ScalarE, fused bias)
    out[t, cc] = sum_{f in half} h[t, f] * W2[f, cc]   (PE, bf16 h, fp32 acc)
out ships to HBM in bf16 (error contribution ~4e-4 rel, budget is 2e-2).

Schedule notes (from perfetto trace of the v1 kernel):
  * The NEFF preamble occupies ~6.4us; the PE steady state is ~99.6% busy at
    the warm N=512 issue rate (216 ns/MM), so the only recoverable time is
    startup (first MM was at 20.3us), the HAM cold-clock window, and the tail.
  * v2: ~80 tiny scratch matmuls issue at t~6.5us to warm the HAM clock gate
    while DMAs stream; the first token chunk is 256 wide and its weight/x
    DMAs are issued in exact consumption order as a few large transfers
    (x is shipped as [128, 8, nt] so a whole chunk is ONE strided DMA).
  * All DMA lands on one FIFO HW queue, so issue order == service order;
    big slot-B weight loads are spread between slot-A chunk loads.
"""

import os

import numpy as np
import ml_dtypes

import concourse.bass as bass
import concourse.mybir as mybir
import concourse.tile as tile
from concourse import bacc
from concourse.bass_utils import run_bass_kernel_spmd

C = 1024
F = 4096
FH = F // 2  # per-core F half
E = 8
K = 2
N_CORES = 8

BF16 = mybir.dt.bfloat16
F32 = mybir.dt.float32

N_CT = C // 128  # 8 contraction tiles for x @ W1
N_FT = FH // 128  # 16 F tiles per half
N_CC = C // 512  # 2 output column chunks


def build_nc(chunks_a: list[int], chunks_b: list[int]) -> bass.Bass:
    """Bass program: two experts' FFNs (F-half depth) over their token chunks."""
    nta, ntb = sum(chunks_a), sum(chunks_b)
    assert all(0 < ch <= 512 for ch in chunks_a + chunks_b)
    nc = bacc.Bacc(None)

    # x shipped pre-swizzled as [128, 8, nt]: (p, chi, t) = xT[chi*128 + p, t]
    # x0a duplicates chunk 0 of slot A contiguously: strided 512B-run HBM
    # reads measured ~60GB/s, contiguous ~350GB/s — and chunk 0 is the
    # startup critical path.
    x0a = nc.dram_tensor("x0a", [128, N_CT, chunks_a[0]], BF16, kind="ExternalInput")
    xta = nc.dram_tensor("xta", [128, N_CT, nta], BF16, kind="ExternalInput")
    xtb = nc.dram_tensor("xtb", [128, N_CT, ntb], BF16, kind="ExternalInput")
    # W1 quarter-major [4, 128, 8, FH/4] so each startup piece is contiguous:
    # (q, p, chi, f) = W1[chi*128 + p, q*512 + f]
    w1a = nc.dram_tensor("w1a", [4, 128, N_CT, FH // 4], BF16, kind="ExternalInput")
    w1b = nc.dram_tensor("w1b", [4, 128, N_CT, FH // 4], BF16, kind="ExternalInput")
    # W2 half-major [2, 128, 8, C]: (h, p, fhi, c) = W2[(8h + fhi)*128 + p, c]
    w2a = nc.dram_tensor("w2a", [2, 128, N_FT // 2, C], BF16, kind="ExternalInput")
    w2b = nc.dram_tensor("w2b", [2, 128, N_FT // 2, C], BF16, kind="ExternalInput")
    # b1t[s][p, j] = b1[slot s][(j*128)+p] for this core's F-half (j: f-tile)
    b1t = nc.dram_tensor("b1t", [2, 128, N_FT], F32, kind="ExternalInput")
    outa = nc.dram_tensor("outa", [nta, C], BF16, kind="ExternalOutput")
    outb = nc.dram_tensor("outb", [ntb, C], BF16, kind="ExternalOutput")

    with tile.TileContext(nc) as tc:
        with (
            tc.tile_pool(name="wpool", bufs=1) as wpool,
            tc.tile_pool(name="xpool", bufs=4) as xpool,
            tc.tile_pool(name="hpool", bufs=N_FT + 2) as hpool,
            tc.tile_pool(name="opool", bufs=4) as opool,
            tc.tile_pool(name="phpool", bufs=4, space="PSUM") as phpool,
            tc.tile_pool(name="popool", bufs=4, space="PSUM") as popool,
        ):
            # --- HAM warmup: N=512 scratch matmuls keep the PE busy from the
            # end of the NEFF preamble (~6.9us) until the first chunk's DMAs
            # land (~13us), so the clock gate opens (K=8/8) at ~10.3us and
            # never re-throttles (an idle >3.4us would drop it back to 1/2
            # clock).  memset on GpSimd: its preamble ends ~3us before the
            # PE's, so the PE never waits.  ~8 cold + ~18 warm MMs = 7.3us.
            warm_w = wpool.tile([128, 512], BF16, name="warm_w", tag="warmw")
            nc.gpsimd.memset(warm_w, 0)
            warm_ps = popool.tile([128, 512], F32, name="warm_ps", tag="po")
            for _ in range(26):
                nc.tensor.matmul(warm_ps[:64, :], lhsT=warm_w[:, :64], rhs=warm_w,
                                 start=True, stop=True)

            # --- SBUF weight/bias tiles
            w1_sb = {
                s: wpool.tile([128, N_CT, FH], BF16, name=f"w1sb{s}", tag=f"w1sb{s}")
                for s in range(2)
            }
            w2_sb = {
                s: wpool.tile([128, N_FT, C], BF16, name=f"w2sb{s}", tag=f"w2sb{s}")
                for s in range(2)
            }
            b1_sb = wpool.tile([128, 2, N_FT], F32, name="b1sb", tag="b1sb")

            # --- Startup DMAs in exact consumption order (single FIFO queue),
            # every piece a fully contiguous HBM read: chunk-0 (256 tokens)
            # runs DMA-paced right behind them.
            QF = FH // 4  # 512: f-columns per W1 quarter

            def load_w1(s, src, q):
                nc.sync.dma_start(
                    out=w1_sb[s][:, :, q * QF : (q + 1) * QF], in_=src[q]
                )

            def load_w2(s, src, h):
                nc.sync.dma_start(out=w2_sb[s][:, 8 * h : 8 * h + 8, :], in_=src[h])

            load_w1(0, w1a, 0)
            xts = {}
            xts[(0, 0)] = xpool.tile(
                [128, N_CT, chunks_a[0]], BF16, name="xta_0", tag="xt"
            )
            nc.sync.dma_start(out=xts[(0, 0)], in_=x0a[:, :, :])
            nc.sync.dma_start(out=b1_sb[:, 0, :], in_=b1t[0])
            nc.sync.dma_start(out=b1_sb[:, 1, :], in_=b1t[1])
            for q in (1, 2, 3):
                load_w1(0, w1a, q)
            load_w2(0, w2a, 0)
            load_w2(0, w2a, 1)

            # slot-B weight loads, spread between slot-A chunks so they don't
            # starve the slot-A activation stream in the FIFO DMA queue.
            deferred_loads = [
                [lambda: load_w1(1, w1b, 0), lambda: load_w1(1, w1b, 1)],
                [lambda: load_w1(1, w1b, 2), lambda: load_w1(1, w1b, 3)],
                [lambda: load_w2(1, w2b, 0)],
                [lambda: load_w2(1, w2b, 1)],
            ]

            def run_slot(s, xtd, outd, chunks):
                tok0 = 0
                for tk, ch in enumerate(chunks):
                    if (s, tk) in xts:
                        xt = xts[(s, tk)]
                    else:
                        xt = xpool.tile(
                            [128, N_CT, ch], BF16, name=f"xt{s}_{tk}", tag="xt"
                        )
                        nc.sync.dma_start(
                            out=xt, in_=xtd[:, :, tok0 : tok0 + ch]
                        )
                    if s == 0 and tk >= 1 and deferred_loads:
                        for emit in deferred_loads.pop(0):
                            emit()

                    hts = []
                    for f in range(N_FT):
                        ph = phpool.tile([128, ch], F32, name=f"ph{s}_{tk}_{f}", tag="ph")
                        for c in range(N_CT):
                            nc.tensor.matmul(
                                ph,
                                lhsT=w1_sb[s][:, c, f * 128 : (f + 1) * 128],
                                rhs=xt[:, c, :],
                                start=(c == 0),
                                stop=(c == N_CT - 1),
                            )
                        ht = hpool.tile([128, ch], BF16, name=f"ht{s}_{tk}_{f}", tag="ht")
                        nc.scalar.activation(
                            out=ht,
                            in_=ph,
                            func=mybir.ActivationFunctionType.Gelu,
                            bias=b1_sb[:, s, f : f + 1],
                            scale=1.0,
                        )
                        hts.append(ht)

                    for tt in range((ch + 127) // 128):
                        tw = min(128, ch - tt * 128)
                        for cc in range(N_CC):
                            po = popool.tile(
                                [128, 512], F32, name=f"po{s}_{tk}_{tt}_{cc}", tag="po"
                            )
                            for f in range(N_FT):
                                nc.tensor.matmul(
                                    po[:tw, :],
                                    lhsT=hts[f][:, tt * 128 : tt * 128 + tw],
                                    rhs=w2_sb[s][:, f, cc * 512 : (cc + 1) * 512],
                                    start=(f == 0),
                                    stop=(f == N_FT - 1),
                                )
                            ot = opool.tile(
                                [128, 512], BF16, name=f"ot{s}_{tk}_{tt}_{cc}", tag="ot"
                            )
                            nc.vector.tensor_copy(ot[:tw, :], po[:tw, :])
                            r0 = tok0 + tt * 128
                            nc.sync.dma_start(
                                out=outd[r0 : r0 + tw, cc * 512 : (cc + 1) * 512],
                                in_=ot[:tw, :],
                            )
                    tok0 += ch

            run_slot(0, xta, outa, chunks_a)
            while deferred_loads:  # in case slot A had very few chunks
                for emit in deferred_loads.pop(0):
                    emit()
            run_slot(1, xtb, outb, chunks_b)
    nc.finalize()
    return nc


def pick_chunks(n: int, small_first: bool) -> list[int]:
    """Chunks <=512 summing to n.  Greedy 512s + tail keeps sum(ceil(ch/128))
    (the MM2 tile count) at its minimum ceil(n/128); a tail <128 is widened by
    borrowing 128 from the previous chunk so MM1 never goes LDWEIGHTS-bound.
    small_first: lead with a 256 chunk so the first chunk's DMAs are small."""
    if n <= 512:
        return [n]
    chunks = []
    rem = n
    if small_first and n > 768:
        chunks.append(256)
        rem -= 256
    while rem > 512:
        chunks.append(512)
        rem -= 512
    if rem < 128 and chunks and chunks[-1] == 512:
        chunks[-1] = 384
        rem += 128
    chunks.append(rem)
    return chunks


def _route(x2d: np.ndarray, Wg: np.ndarray):
    """fp32 gate identical in selection to the reference; returns per-expert
    token indices and renormalized top-2 weights."""
    logits = x2d @ Wg  # fp32 BLAS
    order = np.argsort(-logits, axis=1, kind="stable")
    top2 = order[:, :K]  # [N, 2]
    m = logits.max(axis=1, keepdims=True)
    p = np.exp(logits - m, dtype=np.float32)
    p /= p.sum(axis=1, keepdims=True)
    tw = np.take_along_axis(p, top2, axis=1)
    tw /= tw.sum(axis=1, keepdims=True)  # [N, 2] renormalized
    idxs, ws = [], []
    for e in range(E):
        sel = top2 == e  # [N, 2] bool, at most one True per row
        rows = np.where(sel.any(axis=1))[0]
        idxs.append(rows)
        ws.append(tw[rows][sel[rows]])
    return idxs, ws


_LAST_RESULTS = {}  # stash for test harness introspection (exec time etc.)


def kernel(**inputs: np.ndarray) -> np.ndarray:
    x = np.asarray(inputs["x"], dtype=np.float32)
    Wg = np.asarray(inputs["Wg"], dtype=np.float32)
    W1 = np.asarray(inputs["W1"], dtype=np.float32)
    b1 = np.asarray(inputs["b1"], dtype=np.float32)
    W2 = np.asarray(inputs["W2"], dtype=np.float32)
    b2 = np.asarray(inputs["b2"], dtype=np.float32)

    B, T, Cx = x.shape
    assert Cx == C
    x2d = np.ascontiguousarray(x.reshape(-1, C))
    n_tok_total = x2d.shape[0]

    idxs, ws = _route(x2d, Wg)
    counts = np.array([len(i) for i in idxs])

    # Pair the largest expert with the smallest, 2nd largest with 2nd
    # smallest, etc.  Pair p runs on cores 2p (F-half 0) and 2p+1 (F-half 1).
    # Device slot A (runs first) gets the SMALL expert of each pair: its
    # first chunk is the startup-critical DMA, and slot B (the big experts)
    # then ends the kernel on its small tail chunk, minimizing the final
    # output-DMA drain.
    order = np.argsort(-counts, kind="stable")
    pairs = [(int(order[E - 1 - p]), int(order[p])) for p in range(E // 2)]
    nta = max(counts[a] for a, _ in pairs)
    ntb = max(counts[b] for _, b in pairs)
    chunks_a = pick_chunks(int(nta), small_first=True)
    chunks_b = pick_chunks(int(ntb), small_first=False)
    nta, ntb = sum(chunks_a), sum(chunks_b)

    w1h = W1.astype(ml_dtypes.bfloat16)  # [E, C, F]
    w2h = W2.astype(ml_dtypes.bfloat16)  # [E, F, C]

    def xt_for(e, ntok):
        # [128, 8, ntok] with (p, chi, t) = x[t, chi*128 + p]
        xe = np.zeros((ntok, C), dtype=np.float32)
        xe[: counts[e]] = x2d[idxs[e]]
        xt = xe.T.reshape(N_CT, 128, ntok).transpose(1, 0, 2)
        return np.ascontiguousarray(xt).astype(ml_dtypes.bfloat16)

    xt_cache = {}
    for a, b_ in pairs:
        xt_cache[a] = xt_for(a, nta)
        xt_cache[b_] = xt_for(b_, ntb)

    in_maps = []
    for core in range(N_CORES):
        p, h = divmod(core, 2)
        ea, eb = pairs[p]
        fsl = slice(h * FH, (h + 1) * FH)
        b1t = np.stack(
            [
                np.ascontiguousarray(b1[ea][fsl].reshape(N_FT, 128).T),
                np.ascontiguousarray(b1[eb][fsl].reshape(N_FT, 128).T),
            ]
        ).astype(np.float32)

        def w1_lay(e):  # [4, 128, 8, FH/4]: (q, p, chi, f) = W1[e][chi*128+p, f0+q*512+f]
            w = w1h[e][:, fsl].reshape(N_CT, 128, 4, FH // 4).transpose(2, 1, 0, 3)
            return np.ascontiguousarray(w)

        def w2_lay(e):  # [2, 128, 8, C]: (h, p, fhi, c) = W2[e][f0+(8h+fhi)*128+p, c]
            w = w2h[e][fsl, :].reshape(2, N_FT // 2, 128, C).transpose(0, 2, 1, 3)
            return np.ascontiguousarray(w)

        in_maps.append(
            {
                "x0a": np.ascontiguousarray(xt_cache[ea][:, :, : chunks_a[0]]),
                "xta": xt_cache[ea],
                "xtb": xt_cache[eb],
                "w1a": w1_lay(ea),
                "w1b": w1_lay(eb),
                "w2a": w2_lay(ea),
                "w2b": w2_lay(eb),
                "b1t": b1t,
            }
        )

    nc = build_nc(chunks_a, chunks_b)
    trace = os.environ.get("KERNEL_TRACE", "") == "1"
    res = run_bass_kernel_spmd(
        nc, in_maps, core_ids=list(range(N_CORES)), trace=trace
    )
    _LAST_RESULTS["bass_results"] = res
    if trace and res.exec_time_ns is not None:
        print(f"[kernel] HW exec time: {res.exec_time_ns} ns")

    out = np.zeros((n_tok_total, C), dtype=np.float32)
    for p, (ea, eb) in enumerate(pairs):
        for e, key in ((ea, "outa"), (eb, "outb")):
            n_e = counts[e]
            oe = (
                np.asarray(res.results[2 * p][key])[:n_e].astype(np.float32)
                + np.asarray(res.results[2 * p + 1][key])[:n_e].astype(np.float32)
            )
            out[idxs[e]] += ws[e][:, None] * (oe + b2[e][None, :])
    return out.reshape(B, T, C)


# revision 12
# speedup vs baseline: 1.0383x; 1.0014x over previous
"""Trainium2 Bass kernel for an 8-expert top-2 MoE layer (B=4, T=2048, C=1024,
F=4096), expert-parallel across 8 NeuronCores — quarter-slot balanced variant.

Routing/pairing
---------------
Host computes the fp32 gate and routes each token to its top-2 experts.  The
16 (expert, F-half) jobs of the paired scheme padded every core to
max(bigs)+max(smalls) = 4204 token-half-units (ideal 4096).  Here each core
instead runs FOUR slots at F-QUARTER depth.  Sorted by count, experts are
paired (E1,E2),(E3,E4),(E5,E6),(E7,E8); slot s is compiled for
S_s = max count of pair s, and core i's slot s holds (expert = pair_s[i//4],
F-quarter = i%4).  Every expert-quarter combo lands on exactly one core, and
per-core padded work drops to Sigma S_s = 4139 half-units.

Host sums the 4 per-quarter partial outputs of each expert and scatter-adds
with the gate weights (plus w*b2, which never goes to device).

On-device per slot (expert e, quarter q), streaming token chunks:
    hT[f, t]   = sum_c W1[c, f] * xT[c, t]     f in quarter q   (PE bf16)
    hT         = gelu_erf(hT + b1[f])          (ScalarE, fused bias)
    out[t, cc] = sum_f h[t, f] * W2[f, cc]     (PE bf16, fp32 acc)
out ships bf16.  Startup: scratch-matmul HAM warmup bridging the NEFF
preamble to the first chunk's (contiguous) DMAs; all DMA is one FIFO HW
queue so issue order == service order.
"""

import os

import numpy as np
import ml_dtypes

import concourse.bass as bass
import concourse.mybir as mybir
import concourse.tile as tile
from concourse import bacc
from concourse.bass_utils import run_bass_kernel_spmd

C = 1024
F = 4096
FQ = F // 4  # per-slot F quarter (1024)
E = 8
K = 2
N_CORES = 8
N_SLOTS = 4

BF16 = mybir.dt.bfloat16
F32 = mybir.dt.float32

N_CT = C // 128  # 8 contraction tiles for x @ W1
N_FT = FQ // 128  # 8 F tiles per quarter
N_CC = C // 512  # 2 output column chunks


def build_nc(chunk_lists: list[list[int]]) -> bass.Bass:
    """Bass program: four expert-quarter FFN slots over their token chunks."""
    assert len(chunk_lists) == N_SLOTS
    nts = [sum(chs) for chs in chunk_lists]
    nc = bacc.Bacc(None)

    # per-slot tensors; x pre-swizzled [128, 8, nt]: (p, chi, t) = xT[chi*128+p, t]
    x0 = nc.dram_tensor("x0", [128, N_CT, chunk_lists[0][0]], BF16,
                        kind="ExternalInput")  # contiguous startup copy
    xs = [nc.dram_tensor(f"x{s}t", [128, N_CT, nts[s]], BF16, kind="ExternalInput")
          for s in range(N_SLOTS)]
    # W1 quarter-of-slot pieces [2, 128, 8, FQ/2]: (j, p, chi, f) =
    # W1[chi*128+p, fq0 + j*512 + f] — each piece a contiguous 1MB read.
    w1s = [nc.dram_tensor(f"w1{s}", [2, 128, N_CT, FQ // 2], BF16,
                          kind="ExternalInput") for s in range(N_SLOTS)]
    # W2 halves [2, 128, 4, C]: (h, p, fhi, c) = W2[fq0 + (4h+fhi)*128 + p, c]
    w2s = [nc.dram_tensor(f"w2{s}", [2, 128, N_FT // 2, C], BF16,
                          kind="ExternalInput") for s in range(N_SLOTS)]
    b1t = nc.dram_tensor("b1t", [N_SLOTS, 128, N_FT], F32, kind="ExternalInput")
    outs = [nc.dram_tensor(f"out{s}", [nts[s], C], BF16, kind="ExternalOutput")
            for s in range(N_SLOTS)]

    with tile.TileContext(nc) as tc:
        with (
            tc.tile_pool(name="wpool", bufs=1) as wpool,
            tc.tile_pool(name="xpool", bufs=4) as xpool,
            tc.tile_pool(name="hpool", bufs=N_FT + 2) as hpool,
            tc.tile_pool(name="opool", bufs=4) as opool,
            tc.tile_pool(name="phpool", bufs=4, space="PSUM") as phpool,
            tc.tile_pool(name="popool", bufs=4, space="PSUM") as popool,
        ):
            # HAM warmup: bridge the NEFF preamble (~6.9us) to the first
            # chunk's DMA arrival (~13us) with scratch matmuls so the PE
            # clock gate opens and stays open.
            warm_w = wpool.tile([128, 512], BF16, name="warm_w", tag="warmw")
            nc.gpsimd.memset(warm_w, 0)
            warm_ps = popool.tile([128, 512], F32, name="warm_ps", tag="po")
            for _ in range(20):
                nc.tensor.matmul(warm_ps[:64, :], lhsT=warm_w[:, :64], rhs=warm_w,
                                 start=True, stop=True)

            w1_sb = [wpool.tile([128, N_CT, FQ], BF16, name=f"w1sb{s}", tag=f"w1sb{s}")
                     for s in range(N_SLOTS)]
            w2_sb = [wpool.tile([128, N_FT, C], BF16, name=f"w2sb{s}", tag=f"w2sb{s}")
                     for s in range(N_SLOTS)]
            b1_sb = wpool.tile([128, N_SLOTS, N_FT], F32, name="b1sb", tag="b1sb")

            def load_w1(s, j):
                nc.sync.dma_start(
                    out=w1_sb[s][:, :, j * (FQ // 2) : (j + 1) * (FQ // 2)],
                    in_=w1s[s][j],
                )

            def load_w2(s, h):
                nc.sync.dma_start(
                    out=w2_sb[s][:, 4 * h : 4 * h + 4, :], in_=w2s[s][h]
                )

            # startup: slot-0 criticals in consumption order
            load_w1(0, 0)
            xts = {}
            xts[(0, 0)] = xpool.tile(
                [128, N_CT, chunk_lists[0][0]], BF16, name="x0_0", tag="xt"
            )
            nc.sync.dma_start(out=xts[(0, 0)], in_=x0[:, :, :])
            for s in range(N_SLOTS):
                nc.sync.dma_start(out=b1_sb[:, s, :], in_=b1t[s])
            load_w1(0, 1)
            load_w2(0, 0)
            load_w2(0, 1)

            # later slots' weights: drip between earlier chunks (FIFO queue)
            deferred = []
            for s in range(1, N_SLOTS):
                deferred.append([lambda s=s: load_w1(s, 0)])
                deferred.append([lambda s=s: load_w1(s, 1)])
                deferred.append([lambda s=s: load_w2(s, 0),
                                 lambda s=s: load_w2(s, 1)])

            def run_slot(s):
                chunks = chunk_lists[s]
                tok0 = 0
                for tk, ch in enumerate(chunks):
                    if (s, tk) in xts:
                        xt = xts[(s, tk)]
                    else:
                        xt = xpool.tile(
                            [128, N_CT, ch], BF16, name=f"xt{s}_{tk}", tag="xt"
                        )
                        nc.sync.dma_start(out=xt, in_=xs[s][:, :, tok0 : tok0 + ch])
                    if deferred and not (s == 0 and tk == 0):
                        for emit in deferred.pop(0):
                            emit()

                    hts = []
                    for f in range(N_FT):
                        ph = phpool.tile([128, ch], F32, name=f"ph{s}_{tk}_{f}", tag="ph")
                        for c in range(N_CT):
                            nc.tensor.matmul(
                                ph,
                                lhsT=w1_sb[s][:, c, f * 128 : (f + 1) * 128],
                                rhs=xt[:, c, :],
                                start=(c == 0),
                                stop=(c == N_CT - 1),
                            )
                        ht = hpool.tile([128, ch], BF16, name=f"ht{s}_{tk}_{f}", tag="ht")
                        nc.scalar.activation(
                            out=ht,
                            in_=ph,
                            func=mybir.ActivationFunctionType.Gelu,
                            bias=b1_sb[:, s, f : f + 1],
                            scale=1.0,
                        )
                        hts.append(ht)

                    for tt in range((ch + 127) // 128):
                        tw = min(128, ch - tt * 128)
                        for cc in range(N_CC):
                            po = popool.tile(
                                [128, 512], F32, name=f"po{s}_{tk}_{tt}_{cc}", tag="po"
                            )
                            for f in range(N_FT):
                                nc.tensor.matmul(
                                    po[:tw, :],
                                    lhsT=hts[f][:, tt * 128 : tt * 128 + tw],
                                    rhs=w2_sb[s][:, f, cc * 512 : (cc + 1) * 512],
                                    start=(f == 0),
                                    stop=(f == N_FT - 1),
                                )
                            ot = opool.tile(
                                [128, 512], BF16, name=f"ot{s}_{tk}_{tt}_{cc}", tag="ot"
                            )
                            nc.vector.tensor_copy(ot[:tw, :], po[:tw, :])
                            r0 = tok0 + tt * 128
                            nc.sync.dma_start(
                                out=outs[s][r0 : r0 + tw, cc * 512 : (cc + 1) * 512],
                                in_=ot[:tw, :],
                            )
                    tok0 += ch

            for s in range(N_SLOTS):
                run_slot(s)
            while deferred:
                for emit in deferred.pop(0):
                    emit()
    nc.finalize()
    return nc


def pick_chunks(n: int, small_first: bool) -> list[int]:
    """Chunks <=512 summing to n with sum(ceil(ch/128)) = ceil(n/128); a tail
    <128 borrows 128 from the previous chunk (keeps MM1 off the LDW floor)."""
    if n <= 512:
        return [n]
    chunks = []
    rem = n
    if small_first and n > 768:
        chunks.append(256)
        rem -= 256
    while rem > 512:
        chunks.append(512)
        rem -= 512
    if rem < 128 and chunks and chunks[-1] == 512:
        chunks[-1] = 384
        rem += 128
    chunks.append(rem)
    return chunks


def _route(x2d: np.ndarray, Wg: np.ndarray):
    logits = x2d @ Wg  # fp32 BLAS
    order = np.argsort(-logits, axis=1, kind="stable")
    top2 = order[:, :K]
    m = logits.max(axis=1, keepdims=True)
    p = np.exp(logits - m, dtype=np.float32)
    p /= p.sum(axis=1, keepdims=True)
    tw = np.take_along_axis(p, top2, axis=1)
    tw /= tw.sum(axis=1, keepdims=True)
    idxs, ws = [], []
    for e in range(E):
        sel = top2 == e
        rows = np.where(sel.any(axis=1))[0]
        idxs.append(rows)
        ws.append(tw[rows][sel[rows]])
    return idxs, ws


_LAST_RESULTS = {}


def kernel(**inputs: np.ndarray) -> np.ndarray:
    x = np.asarray(inputs["x"], dtype=np.float32)
    Wg = np.asarray(inputs["Wg"], dtype=np.float32)
    W1 = np.asarray(inputs["W1"], dtype=np.float32)
    b1 = np.asarray(inputs["b1"], dtype=np.float32)
    W2 = np.asarray(inputs["W2"], dtype=np.float32)
    b2 = np.asarray(inputs["b2"], dtype=np.float32)

    B, T, Cx = x.shape
    assert Cx == C
    x2d = np.ascontiguousarray(x.reshape(-1, C))
    n_tok_total = x2d.shape[0]

    idxs, ws = _route(x2d, Wg)
    counts = np.array([len(i) for i in idxs])

    # slot s serves the adjacent sorted pair (E_{2s}, E_{2s+1}); compiled
    # token count S_s = the larger of the two.  Slot 0 = smallest pair (its
    # chunk 0 is the startup critical path); slot 3 = largest pair but
    # reversed so the kernel still ends on a small tail chunk.
    order = np.argsort(-counts, kind="stable")
    slot_pairs = [
        (int(order[2 * s]), int(order[2 * s + 1])) for s in range(N_SLOTS)
    ][::-1]  # slot 0 = smallest counts, slot 3 = largest
    S = [int(max(counts[a], counts[b])) for a, b in slot_pairs]
    chunk_lists = [pick_chunks(S[s], small_first=(s == 0)) for s in range(N_SLOTS)]
    S = [sum(chs) for chs in chunk_lists]

    w1h = W1.astype(ml_dtypes.bfloat16)  # [E, C, F]
    w2h = W2.astype(ml_dtypes.bfloat16)  # [E, F, C]

    def xt_for(e, ntok):
        xe = np.zeros((ntok, C), dtype=np.float32)
        xe[: counts[e]] = x2d[idxs[e]]
        xt = xe.T.reshape(N_CT, 128, ntok).transpose(1, 0, 2)
        return np.ascontiguousarray(xt).astype(ml_dtypes.bfloat16)

    xt_cache = {}
    for s, (a, b_) in enumerate(slot_pairs):
        for e in (a, b_):
            xt_cache[e] = xt_for(e, S[s])

    in_maps = []
    for core in range(N_CORES):
        q = core % 4  # this core's F-quarter
        fsl = slice(q * FQ, (q + 1) * FQ)
        im = {}
        b1rows = []
        for s in range(N_SLOTS):
            e = slot_pairs[s][core // 4]  # expert for this core's slot s
            # W1 [2, 128, 8, FQ/2]
            w = w1h[e][:, fsl].reshape(N_CT, 128, 2, FQ // 2).transpose(2, 1, 0, 3)
            im[f"w1{s}"] = np.ascontiguousarray(w)
            # W2 [2, 128, 4, C]
            w = w2h[e][fsl, :].reshape(2, N_FT // 2, 128, C).transpose(0, 2, 1, 3)
            im[f"w2{s}"] = np.ascontiguousarray(w)
            im[f"x{s}t"] = xt_cache[e]
            b1rows.append(
                np.ascontiguousarray(b1[e][fsl].reshape(N_FT, 128).T)
            )
        im["b1t"] = np.stack(b1rows).astype(np.float32)
        im["x0"] = np.ascontiguousarray(
            im["x0t"][:, :, : chunk_lists[0][0]]
        )
        in_maps.append(im)

    nc = build_nc(chunk_lists)
    trace = os.environ.get("KERNEL_TRACE", "") == "1"
    res = run_bass_kernel_spmd(
        nc, in_maps, core_ids=list(range(N_CORES)), trace=trace
    )
    _LAST_RESULTS["bass_results"] = res
    if trace and res.exec_time_ns is not None:
        print(f"[kernel] HW exec time: {res.exec_time_ns} ns")

    out = np.zeros((n_tok_total, C), dtype=np.float32)
    for s in range(N_SLOTS):
        for half, e in enumerate(slot_pairs[s]):
            n_e = counts[e]
            oe = np.zeros((n_e, C), dtype=np.float32)
            for q in range(4):
                core = 4 * half + q
                oe += np.asarray(res.results[core][f"out{s}"])[:n_e].astype(
                    np.float32
                )
            out[idxs[e]] += ws[e][:, None] * (oe + b2[e][None, :])
    return out.reshape(B, T, C)


# revision 13
# speedup vs baseline: 1.0393x; 1.0009x over previous
"""Trainium2 Bass kernel for an 8-expert top-2 MoE layer (B=4, T=2048, C=1024,
F=4096), expert-parallel across 8 NeuronCores — quarter-slot balanced variant.

Routing/pairing
---------------
Host computes the fp32 gate and routes each token to its top-2 experts.  The
16 (expert, F-half) jobs of the paired scheme padded every core to
max(bigs)+max(smalls) = 4204 token-half-units (ideal 4096).  Here each core
instead runs FOUR slots at F-QUARTER depth.  Sorted by count, experts are
paired (E1,E2),(E3,E4),(E5,E6),(E7,E8); slot s is compiled for
S_s = max count of pair s, and core i's slot s holds (expert = pair_s[i//4],
F-quarter = i%4).  Every expert-quarter combo lands on exactly one core, and
per-core padded work drops to Sigma S_s = 4139 half-units.

Host sums the 4 per-quarter partial outputs of each expert and scatter-adds
with the gate weights (plus w*b2, which never goes to device).

On-device per slot (expert e, quarter q), streaming token chunks:
    hT[f, t]   = sum_c W1[c, f] * xT[c, t]     f in quarter q   (PE bf16)
    hT         = gelu_erf(hT + b1[f])          (ScalarE, fused bias)
    out[t, cc] = sum_f h[t, f] * W2[f, cc]     (PE bf16, fp32 acc)
out ships bf16.  Startup: scratch-matmul HAM warmup bridging the NEFF
preamble to the first chunk's (contiguous) DMAs; all DMA is one FIFO HW
queue so issue order == service order.
"""

import os

import numpy as np
import ml_dtypes

import concourse.bass as bass
import concourse.mybir as mybir
import concourse.tile as tile
from concourse import bacc
from concourse.bass_utils import run_bass_kernel_spmd

C = 1024
F = 4096
FQ = F // 4  # per-slot F quarter (1024)
E = 8
K = 2
N_CORES = 8
N_SLOTS = 4

BF16 = mybir.dt.bfloat16
F32 = mybir.dt.float32

N_CT = C // 128  # 8 contraction tiles for x @ W1
N_FT = FQ // 128  # 8 F tiles per quarter
N_CC = C // 512  # 2 output column chunks


def build_nc(chunk_lists: list[list[int]]) -> bass.Bass:
    """Bass program: four expert-quarter FFN slots over their token chunks."""
    assert len(chunk_lists) == N_SLOTS
    nts = [sum(chs) for chs in chunk_lists]
    nc = bacc.Bacc(None)

    # per-slot tensors; x pre-swizzled [128, 8, nt]: (p, chi, t) = xT[chi*128+p, t]
    x0 = nc.dram_tensor("x0", [128, N_CT, chunk_lists[0][0]], BF16,
                        kind="ExternalInput")  # contiguous startup copy
    xs = [nc.dram_tensor(f"x{s}t", [128, N_CT, nts[s]], BF16, kind="ExternalInput")
          for s in range(N_SLOTS)]
    # W1 pieces [4, 128, 8, FQ/4]: (j, p, chi, f) = W1[chi*128+p,
    # fq0 + j*(FQ/4) + f] — each piece a contiguous 0.5MB read, so the
    # startup-critical first piece lands (and unblocks MM1 f0-f1) sooner.
    w1s = [nc.dram_tensor(f"w1{s}", [4, 128, N_CT, FQ // 4], BF16,
                          kind="ExternalInput") for s in range(N_SLOTS)]
    # W2 halves [2, 128, 4, C]: (h, p, fhi, c) = W2[fq0 + (4h+fhi)*128 + p, c]
    w2s = [nc.dram_tensor(f"w2{s}", [2, 128, N_FT // 2, C], BF16,
                          kind="ExternalInput") for s in range(N_SLOTS)]
    b1t = nc.dram_tensor("b1t", [N_SLOTS, 128, N_FT], F32, kind="ExternalInput")
    outs = [nc.dram_tensor(f"out{s}", [nts[s], C], BF16, kind="ExternalOutput")
            for s in range(N_SLOTS)]

    with tile.TileContext(nc) as tc:
        with (
            tc.tile_pool(name="wpool", bufs=1) as wpool,
            tc.tile_pool(name="xpool", bufs=4) as xpool,
            tc.tile_pool(name="hpool", bufs=N_FT + 2) as hpool,
            tc.tile_pool(name="opool", bufs=4) as opool,
            tc.tile_pool(name="phpool", bufs=4, space="PSUM") as phpool,
            tc.tile_pool(name="popool", bufs=4, space="PSUM") as popool,
        ):
            # HAM warmup: bridge the NEFF preamble (~6.9us) to the first
            # chunk's DMA arrival (~13us) with scratch matmuls so the PE
            # clock gate opens and stays open.
            warm_w = wpool.tile([128, 512], BF16, name="warm_w", tag="warmw")
            nc.gpsimd.memset(warm_w, 0)
            warm_ps = popool.tile([128, 512], F32, name="warm_ps", tag="po")
            for _ in range(17):
                nc.tensor.matmul(warm_ps[:64, :], lhsT=warm_w[:, :64], rhs=warm_w,
                                 start=True, stop=True)

            w1_sb = [wpool.tile([128, N_CT, FQ], BF16, name=f"w1sb{s}", tag=f"w1sb{s}")
                     for s in range(N_SLOTS)]
            w2_sb = [wpool.tile([128, N_FT, C], BF16, name=f"w2sb{s}", tag=f"w2sb{s}")
                     for s in range(N_SLOTS)]
            b1_sb = wpool.tile([128, N_SLOTS, N_FT], F32, name="b1sb", tag="b1sb")

            def load_w1(s, j):
                nc.sync.dma_start(
                    out=w1_sb[s][:, :, j * (FQ // 4) : (j + 1) * (FQ // 4)],
                    in_=w1s[s][j],
                )

            def load_w2(s, h):
                nc.sync.dma_start(
                    out=w2_sb[s][:, 4 * h : 4 * h + 4, :], in_=w2s[s][h]
                )

            # startup: slot-0 criticals in consumption order
            load_w1(0, 0)
            xts = {}
            xts[(0, 0)] = xpool.tile(
                [128, N_CT, chunk_lists[0][0]], BF16, name="x0_0", tag="xt"
            )
            nc.sync.dma_start(out=xts[(0, 0)], in_=x0[:, :, :])
            for s in range(N_SLOTS):
                nc.sync.dma_start(out=b1_sb[:, s, :], in_=b1t[s])
            for j in (1, 2, 3):
                load_w1(0, j)
            load_w2(0, 0)
            load_w2(0, 1)

            # later slots' weights: drip between earlier chunks (FIFO queue)
            deferred = []
            for s in range(1, N_SLOTS):
                deferred.append([lambda s=s: load_w1(s, 0),
                                 lambda s=s: load_w1(s, 1)])
                deferred.append([lambda s=s: load_w1(s, 2),
                                 lambda s=s: load_w1(s, 3)])
                deferred.append([lambda s=s: load_w2(s, 0),
                                 lambda s=s: load_w2(s, 1)])

            def run_slot(s):
                chunks = chunk_lists[s]
                tok0 = 0
                for tk, ch in enumerate(chunks):
                    if (s, tk) in xts:
                        xt = xts[(s, tk)]
                    else:
                        xt = xpool.tile(
                            [128, N_CT, ch], BF16, name=f"xt{s}_{tk}", tag="xt"
                        )
                        nc.sync.dma_start(out=xt, in_=xs[s][:, :, tok0 : tok0 + ch])
                    if deferred and not (s == 0 and tk == 0):
                        for emit in deferred.pop(0):
                            emit()

                    hts = []
                    for f in range(N_FT):
                        ph = phpool.tile([128, ch], F32, name=f"ph{s}_{tk}_{f}", tag="ph")
                        for c in range(N_CT):
                            nc.tensor.matmul(
                                ph,
                                lhsT=w1_sb[s][:, c, f * 128 : (f + 1) * 128],
                                rhs=xt[:, c, :],
                                start=(c == 0),
                                stop=(c == N_CT - 1),
                            )
                        ht = hpool.tile([128, ch], BF16, name=f"ht{s}_{tk}_{f}", tag="ht")
                        nc.scalar.activation(
                            out=ht,
                            in_=ph,
                            func=mybir.ActivationFunctionType.Gelu,
                            bias=b1_sb[:, s, f : f + 1],
                            scale=1.0,
                        )
                        hts.append(ht)

                    for tt in range((ch + 127) // 128):
                        tw = min(128, ch - tt * 128)
                        for cc in range(N_CC):
                            po = popool.tile(
                                [128, 512], F32, name=f"po{s}_{tk}_{tt}_{cc}", tag="po"
                            )
                            for f in range(N_FT):
                                nc.tensor.matmul(
                                    po[:tw, :],
                                    lhsT=hts[f][:, tt * 128 : tt * 128 + tw],
                                    rhs=w2_sb[s][:, f, cc * 512 : (cc + 1) * 512],
                                    start=(f == 0),
                                    stop=(f == N_FT - 1),
                                )
                            ot = opool.tile(
                                [128, 512], BF16, name=f"ot{s}_{tk}_{tt}_{cc}", tag="ot"
                            )
                            nc.vector.tensor_copy(ot[:tw, :], po[:tw, :])
                            r0 = tok0 + tt * 128
                            nc.sync.dma_start(
                                out=outs[s][r0 : r0 + tw, cc * 512 : (cc + 1) * 512],
                                in_=ot[:tw, :],
                            )
                    tok0 += ch

            for s in range(N_SLOTS):
                run_slot(s)
            while deferred:
                for emit in deferred.pop(0):
                    emit()
    nc.finalize()
    return nc


def pick_chunks(n: int, small_first: bool) -> list[int]:
    """Chunks <=512 summing to n with sum(ceil(ch/128)) = ceil(n/128); a tail
    <128 borrows 128 from the previous chunk (keeps MM1 off the LDW floor)."""
    if n <= 512:
        return [n]
    chunks = []
    rem = n
    if small_first and n > 768:
        chunks.append(256)
        rem -= 256
    while rem > 512:
        chunks.append(512)
        rem -= 512
    if rem < 128 and chunks and chunks[-1] == 512:
        chunks[-1] = 384
        rem += 128
    chunks.append(rem)
    return chunks


def _route(x2d: np.ndarray, Wg: np.ndarray):
    logits = x2d @ Wg  # fp32 BLAS
    order = np.argsort(-logits, axis=1, kind="stable")
    top2 = order[:, :K]
    m = logits.max(axis=1, keepdims=True)
    p = np.exp(logits - m, dtype=np.float32)
    p /= p.sum(axis=1, keepdims=True)
    tw = np.take_along_axis(p, top2, axis=1)
    tw /= tw.sum(axis=1, keepdims=True)
    idxs, ws = [], []
    for e in range(E):
        sel = top2 == e
        rows = np.where(sel.any(axis=1))[0]
        idxs.append(rows)
        ws.append(tw[rows][sel[rows]])
    return idxs, ws


_LAST_RESULTS = {}


def kernel(**inputs: np.ndarray) -> np.ndarray:
    x = np.asarray(inputs["x"], dtype=np.float32)
    Wg = np.asarray(inputs["Wg"], dtype=np.float32)
    W1 = np.asarray(inputs["W1"], dtype=np.float32)
    b1 = np.asarray(inputs["b1"], dtype=np.float32)
    W2 = np.asarray(inputs["W2"], dtype=np.float32)
    b2 = np.asarray(inputs["b2"], dtype=np.float32)

    B, T, Cx = x.shape
    assert Cx == C
    x2d = np.ascontiguousarray(x.reshape(-1, C))
    n_tok_total = x2d.shape[0]

    idxs, ws = _route(x2d, Wg)
    counts = np.array([len(i) for i in idxs])

    # slot s serves the adjacent sorted pair (E_{2s}, E_{2s+1}); compiled
    # token count S_s = the larger of the two.  Slot 0 = smallest pair (its
    # chunk 0 is the startup critical path); slot 3 = largest pair but
    # reversed so the kernel still ends on a small tail chunk.
    order = np.argsort(-counts, kind="stable")
    slot_pairs = [
        (int(order[2 * s]), int(order[2 * s + 1])) for s in range(N_SLOTS)
    ][::-1]  # slot 0 = smallest counts, slot 3 = largest
    S = [int(max(counts[a], counts[b])) for a, b in slot_pairs]
    chunk_lists = [pick_chunks(S[s], small_first=(s == 0)) for s in range(N_SLOTS)]
    S = [sum(chs) for chs in chunk_lists]

    w1h = W1.astype(ml_dtypes.bfloat16)  # [E, C, F]
    w2h = W2.astype(ml_dtypes.bfloat16)  # [E, F, C]

    def xt_for(e, ntok):
        xe = np.zeros((ntok, C), dtype=np.float32)
        xe[: counts[e]] = x2d[idxs[e]]
        xt = xe.T.reshape(N_CT, 128, ntok).transpose(1, 0, 2)
        return np.ascontiguousarray(xt).astype(ml_dtypes.bfloat16)

    xt_cache = {}
    for s, (a, b_) in enumerate(slot_pairs):
        for e in (a, b_):
            xt_cache[e] = xt_for(e, S[s])

    in_maps = []
    for core in range(N_CORES):
        q = core % 4  # this core's F-quarter
        fsl = slice(q * FQ, (q + 1) * FQ)
        im = {}
        b1rows = []
        for s in range(N_SLOTS):
            e = slot_pairs[s][core // 4]  # expert for this core's slot s
            # W1 [2, 128, 8, FQ/2]
            w = w1h[e][:, fsl].reshape(N_CT, 128, 4, FQ // 4).transpose(2, 1, 0, 3)
            im[f"w1{s}"] = np.ascontiguousarray(w)
            # W2 [2, 128, 4, C]
            w = w2h[e][fsl, :].reshape(2, N_FT // 2, 128, C).transpose(0, 2, 1, 3)
            im[f"w2{s}"] = np.ascontiguousarray(w)
            im[f"x{s}t"] = xt_cache[e]
            b1rows.append(
                np.ascontiguousarray(b1[e][fsl].reshape(N_FT, 128).T)
            )
        im["b1t"] = np.stack(b1rows).astype(np.float32)
        im["x0"] = np.ascontiguousarray(
            im["x0t"][:, :, : chunk_lists[0][0]]
        )
        in_maps.append(im)

    nc = build_nc(chunk_lists)
    trace = os.environ.get("KERNEL_TRACE", "") == "1"
    res = run_bass_kernel_spmd(
        nc, in_maps, core_ids=list(range(N_CORES)), trace=trace
    )
    _LAST_RESULTS["bass_results"] = res
    if trace and res.exec_time_ns is not None:
        print(f"[kernel] HW exec time: {res.exec_time_ns} ns")

    out = np.zeros((n_tok_total, C), dtype=np.float32)
    for s in range(N_SLOTS):
        for half, e in enumerate(slot_pairs[s]):
            n_e = counts[e]
            oe = np.zeros((n_e, C), dtype=np.float32)
            for q in range(4):
                core = 4 * half + q
                oe += np.asarray(res.results[core][f"out{s}"])[:n_e].astype(
                    np.float32
                )
            out[idxs[e]] += ws[e][:, None] * (oe + b2[e][None, :])
    return out.reshape(B, T, C)
